# revision 9
# baseline (speedup 1.0000x reference)
"""Fused attention layer (projections + masked softmax + context) on 8 TRN2
NeuronCores, data-parallel over batch (one batch element per core).

Per core (batch b):
  pq = q @ Wq.T + bq ; pk = k @ Wk.T + bk ; pv = v @ Wv.T + bv
  scores = pq @ pk.T / 32 ;  E = exp(scores) * mask
  attn = E / rowsum(E) ;  ctx = attn @ pv

Layout strategy: all matmuls contract over the SBUF partition dim, so
activations/weights are transposed on-chip via TensorE (identity matmul).
Matmul operands are produced as float32r (full PE rate for f32 data at
free-dim >= 256). pqT/pkT are staged through DRAM scratch; pv stays
resident in SBUF. Phase 2 runs per 128-row query panel, software-pipelined
(scores of panel p+1 overlap softmax of panel p).
"""
import os
from contextlib import ExitStack

import numpy as np

import concourse.bass as bass
import concourse.tile as tile
from concourse import bacc, mybir, masks
from concourse.bass_utils import run_bass_kernel_spmd

F32 = mybir.dt.float32
F32R = mybir.dt.float32r
I32 = mybir.dt.int32
AF = mybir.ActivationFunctionType
ALU = mybir.AluOpType

P = 128
B = 8
QN = 2048
KN = 2048
D = 1024           # DIN == DPROJ
DC = D // P        # 8 contraction/partition chunks
NPANEL = QN // P   # 16 query panels
SCALE = 1.0 / 32.0 # 1/sqrt(DPROJ)

N_CORES = 8

_cached_nc = None


def _emit_transpose_in(nc, tc, x_dram, ident, xT, psum_tr, cb_toggle):
    """DMA x [2048(rows), 1024] natural -> transpose -> xT [c_in_cc, cc, rows] f32r."""
    nrows = x_dram.shape[0]
    nchunks = nrows // P          # row chunks of 128
    with tc.tile_pool(name="xnat", bufs=2) as xnat_pool:
        for s2 in range(nchunks // 2):  # panels of 256 rows
            xnat = xnat_pool.tile([P, 2, D], F32)
            nc.sync.dma_start(
                out=xnat[:],
                in_=x_dram[s2 * 256:(s2 + 1) * 256, :].rearrange("(s p) c -> p s c", p=P),
            )
            for cc in range(DC):
                pt = psum_tr.tile([P, 2, P], F32)
                for j in range(2):
                    nc.tensor.transpose(
                        pt[:, j, :], xnat[:, j, cc * P:(cc + 1) * P], ident[:]
                    )
                dst = xT[:, cc, s2 * 256:(s2 + 1) * 256]
                src = pt[:, :, :]
                if cb_toggle[0] % 2 == 0:
                    nc.scalar.activation(dst, src, AF.Copy)
                else:
                    nc.vector.tensor_copy(dst, src)
                cb_toggle[0] += 1


def _build():
    nc = bacc.Bacc("TRN2", target_bir_lowering=False, debug=False,
                   num_devices=N_CORES)

    q_d = nc.dram_tensor("query", [QN, D], F32, kind="ExternalInput").ap()
    k_d = nc.dram_tensor("key", [KN, D], F32, kind="ExternalInput").ap()
    v_d = nc.dram_tensor("value", [KN, D], F32, kind="ExternalInput").ap()
    m_d = nc.dram_tensor("mask", [1, KN], I32, kind="ExternalInput").ap()
    w_q = nc.dram_tensor("Wq", [D, D], F32, kind="ExternalInput").ap()
    b_q = nc.dram_tensor("bq", [1, D], F32, kind="ExternalInput").ap()
    w_k = nc.dram_tensor("Wk", [D, D], F32, kind="ExternalInput").ap()
    b_k = nc.dram_tensor("bk", [1, D], F32, kind="ExternalInput").ap()
    w_v = nc.dram_tensor("Wv", [D, D], F32, kind="ExternalInput").ap()
    b_v = nc.dram_tensor("bv", [1, D], F32, kind="ExternalInput").ap()

    ctx_d = nc.dram_tensor("ctx", [QN, D], F32, kind="ExternalOutput").ap()
    attn_d = nc.dram_tensor("attn", [QN, KN], F32, kind="ExternalOutput").ap()

    # DRAM scratch for the transposed projections of q and k: [dc, d_in_dc, n]
    pqt_d = nc.dram_tensor("pqt_scratch", [DC, P, QN], F32R, kind="Internal").ap()
    pkt_d = nc.dram_tensor("pkt_scratch", [DC, P, KN], F32R, kind="Internal").ap()

    cb_toggle = [0]  # alternate ACT/DVE for PSUM->SBUF copybacks

    with tile.TileContext(nc) as tc, ExitStack() as top:
        const_pool = top.enter_context(tc.tile_pool(name="consts", bufs=1))
        ident = const_pool.tile([P, P], F32)
        masks.make_identity(nc, ident[:])

        mask_f = const_pool.tile([P, KN], F32)
        with tc.tile_pool(name="mtmp", bufs=1) as mtmp:
            mask_i = mtmp.tile([P, KN], I32)
            nc.gpsimd.dma_start(out=mask_i[:], in_=m_d.to_broadcast([P, KN]))
            nc.vector.tensor_copy(mask_f[:], mask_i[:])

        bias_q = const_pool.tile([P, DC], F32)
        nc.gpsimd.dma_start(out=bias_q[:], in_=bass.AP(
            tensor=b_q.tensor, offset=b_q.offset, ap=[[1, P], [P, DC]]))
        bias_k = const_pool.tile([P, DC], F32)
        nc.gpsimd.dma_start(out=bias_k[:], in_=bass.AP(
            tensor=b_k.tensor, offset=b_k.offset, ap=[[1, P], [P, DC]]))
        bv_bc = const_pool.tile([P, D], F32)
        nc.gpsimd.dma_start(out=bv_bc[:], in_=b_v.to_broadcast([P, D]))

        # ph2 spans the v step and phase 2: pv is produced in the v step and
        # consumed by every ctx matmul, so its pool must live across both.
        ph2 = top.enter_context(ExitStack())

        # ---------------- Phase 1: projections ----------------
        def weights_T(tcx, w_dram, psum_tr, wpool):
            """Load W [d, c] natural, transpose -> wt [c_in_cc, cc, d] f32r."""
            wt = wpool.tile([P, DC, D], F32R)
            with tc.tile_pool(name="wnat", bufs=1) as wnp:
                w_nat = wnp.tile([P, DC, D], F32)
                nc.sync.dma_start(
                    out=w_nat[:],
                    in_=w_dram.rearrange("(dc p) c -> p dc c", p=P))
                for cc in range(DC):
                    for dg in range(2):      # groups of 4 d-chunks
                        pt = psum_tr.tile([P, 4, P], F32)
                        for j in range(4):
                            dc = dg * 4 + j
                            nc.tensor.transpose(
                                pt[:, j, :],
                                w_nat[:, dc, cc * P:(cc + 1) * P], ident[:])
                        dst = wt[:, cc, dg * 512:(dg + 1) * 512]
                        if cb_toggle[0] % 2 == 0:
                            nc.scalar.activation(dst, pt[:, :, :], AF.Copy)
                        else:
                            nc.vector.tensor_copy(dst, pt[:, :, :])
                        cb_toggle[0] += 1
            return wt

        # q and k: out [d, n] layout, spilled to DRAM scratch
        for x_dram, w_dram, bias_t, out_scratch in (
                (q_d, w_q, bias_q, pqt_d), (k_d, w_k, bias_k, pkt_d)):
            with ExitStack() as ph:
                psum_tr = ph.enter_context(
                    tc.tile_pool(name="ps_tr", bufs=2, space="PSUM"))
                wpool = ph.enter_context(tc.tile_pool(name="wt", bufs=1))
                wt = weights_T(tc, w_dram, psum_tr, wpool)

                xtp = ph.enter_context(tc.tile_pool(name="xt", bufs=1))
                xT = xtp.tile([P, DC, QN], F32R)
                _emit_transpose_in(nc, tc, x_dram, ident, xT, psum_tr, cb_toggle)

                obp = ph.enter_context(tc.tile_pool(name="ob", bufs=2))
                psum_pr = ph.enter_context(
                    tc.tile_pool(name="ps_pr", bufs=3, space="PSUM"))

                for dc in range(DC):
                    ps_a = psum_pr.tile([P, 2, 512], F32, tag="ps_proj")
                    ps_b = psum_pr.tile([P, 2, 512], F32, tag="ps_proj")
                    for cc in range(DC):
                        for n4 in range(4):
                            pt = ps_a if n4 < 2 else ps_b
                            nc.tensor.matmul(
                                pt[:, n4 % 2, :],
                                lhsT=wt[:, cc, dc * P:(dc + 1) * P],
                                rhs=xT[:, cc, n4 * 512:(n4 + 1) * 512],
                                start=(cc == 0), stop=(cc == DC - 1))
                    ob = obp.tile([P, 4, 512], F32R)
                    for n4 in range(4):
                        pt = ps_a if n4 < 2 else ps_b
                        nc.scalar.activation(
                            ob[:, n4, :], pt[:, n4 % 2, :], AF.Identity,
                            bias=bias_t[:, dc:dc + 1], scale=1.0)
                    nc.sync.dma_start(
                        out=out_scratch[dc, :, :], in_=ob[:, :, :])

        # v: out pv [k, d] natural layout, resident
        with ExitStack() as ph:
            pv_pool = ph2.enter_context(tc.tile_pool(name="pv", bufs=1))
            pv = pv_pool.tile([P, KN // P, D], F32R)   # [k_in_s, s, d]

            psum_tr = ph.enter_context(
                tc.tile_pool(name="ps_tr_v", bufs=2, space="PSUM"))
            wpool = ph.enter_context(tc.tile_pool(name="wvt", bufs=1))
            wvt = weights_T(tc, w_v, psum_tr, wpool)

            xtp = ph.enter_context(tc.tile_pool(name="vt", bufs=1))
            vT = xtp.tile([P, DC, KN], F32R)
            _emit_transpose_in(nc, tc, v_d, ident, vT, psum_tr, cb_toggle)

            psum_pv = ph.enter_context(
                tc.tile_pool(name="ps_pv", bufs=3, space="PSUM"))
            for s in range(KN // P):
                ps_v = psum_pv.tile([P, 2, 512], F32)
                for cc in range(DC):
                    for d2 in range(2):
                        nc.tensor.matmul(
                            ps_v[:, d2, :],
                            lhsT=vT[:, cc, s * P:(s + 1) * P],
                            rhs=wvt[:, cc, d2 * 512:(d2 + 1) * 512],
                            start=(cc == 0), stop=(cc == DC - 1))
                for d2 in range(2):
                    nc.vector.scalar_tensor_tensor(
                        out=pv[:, s, d2 * 512:(d2 + 1) * 512],
                        in0=ps_v[:, d2, :], scalar=1.0,
                        in1=bv_bc[:, d2 * 512:(d2 + 1) * 512],
                        op0=ALU.mult, op1=ALU.add)

        # ---------------- Phase 2: attention, per query panel ----------------
        with ExitStack() as ph:
            pktp = ph.enter_context(tc.tile_pool(name="pkt", bufs=1))
            pqp_pool = ph.enter_context(tc.tile_pool(name="pqp", bufs=2))
            e_pool = ph.enter_context(tc.tile_pool(name="em", bufs=1))
            attn_pool = ph.enter_context(tc.tile_pool(name="attnsb", bufs=2))
            at_pool = ph.enter_context(tc.tile_pool(name="attnT", bufs=1))
            ctx_pool = ph.enter_context(tc.tile_pool(name="ctxsb", bufs=1))
            small = ph.enter_context(tc.tile_pool(name="small", bufs=4))
            ps_s = ph.enter_context(tc.tile_pool(name="ps_s", bufs=2, space="PSUM"))
            ps_t = ph.enter_context(tc.tile_pool(name="ps_t", bufs=2, space="PSUM"))
            ps_c = ph.enter_context(tc.tile_pool(name="ps_c", bufs=1, space="PSUM"))

            pkt = pktp.tile([P, DC, KN], F32R)
            for dc in range(DC):
                nc.sync.dma_start(out=pkt[:, dc, :], in_=pkt_d[dc, :, :])

            pqp = [None] * NPANEL

            def load_pqp(p):
                pqp[p] = pqp_pool.tile([P, DC, P], F32R, name="pqp")
                nc.sync.dma_start(
                    out=pqp[p][:],
                    in_=pqt_d[:, :, p * P:(p + 1) * P].rearrange("a p n -> p a n"))

            def scores_half(p, h):
                ps = ps_s.tile([P, 2, 512], F32)
                for dc in range(DC):
                    for n2 in range(2):
                        nc.tensor.matmul(
                            ps[:, n2, :],
                            lhsT=pqp[p][:, dc, :],
                            rhs=pkt[:, dc, (2 * h + n2) * 512:(2 * h + n2 + 1) * 512],
                            start=(dc == 0), stop=(dc == DC - 1))
                return ps

            # prologue
            load_pqp(0)
            load_pqp(1)
            s_half = [scores_half(0, 0), scores_half(0, 1)]

            for p in range(NPANEL):
                # softmax for panel p (ACT/DVE; overlaps next panel's scores on PE)
                em = e_pool.tile([P, KN], F32)
                for h in range(2):
                    nc.scalar.activation(
                        em[:, h * 1024:(h + 1) * 1024],
                        s_half[h][:, :, :], AF.Exp, scale=SCALE)
                den = small.tile([P, 1], F32)
                nc.vector.scalar_tensor_tensor(
                    out=em[:], in0=em[:], scalar=1.0, in1=mask_f[:],
                    op0=ALU.mult, op1=ALU.mult, accum_out=den[:])
                rden = small.tile([P, 1], F32)
                nc.vector.reciprocal(rden[:], den[:])

                # next panel's first scores half keeps PE busy during softmax
                if p + 1 < NPANEL:
                    if p + 2 < NPANEL:
                        load_pqp(p + 2)
                    ns0 = scores_half(p + 1, 0)

                # attn output row-panel
                attn_sb = attn_pool.tile([P, KN], F32)
                nc.vector.tensor_scalar_mul(attn_sb[:], em[:], rden[:])
                nc.sync.dma_start(
                    out=attn_d[p * P:(p + 1) * P, :], in_=attn_sb[:])

                # transpose Em -> attnT [k_in_s, s, nq] (normalization folded
                # into the ctx copyback scale)
                attnT = at_pool.tile([P, KN // P, P], F32R)
                for g in range(4):
                    pt = ps_t.tile([P, 4, P], F32)
                    for j in range(4):
                        s_idx = g * 4 + j
                        nc.tensor.transpose(
                            pt[:, j, :],
                            em[:, s_idx * P:(s_idx + 1) * P], ident[:])
                    dst = attnT[:, g * 4:(g + 1) * 4, :]
                    if g % 2 == 0:
                        nc.scalar.activation(dst, pt[:, :, :], AF.Copy)
                    else:
                        nc.vector.tensor_copy(dst, pt[:, :, :])

                # ctx panel
                ps_ctx = ps_c.tile([P, 2, 512], F32)
                for s in range(KN // P):
                    for d2 in range(2):
                        nc.tensor.matmul(
                            ps_ctx[:, d2, :],
                            lhsT=attnT[:, s, :],
                            rhs=pv[:, s, d2 * 512:(d2 + 1) * 512],
                            start=(s == 0), stop=(s == KN // P - 1))
                ctx_sb = ctx_pool.tile([P, D], F32)
                nc.scalar.activation(ctx_sb[:], ps_ctx[:, :, :], AF.Copy,
                                     scale=rden[:])
                nc.sync.dma_start(
                    out=ctx_d[p * P:(p + 1) * P, :], in_=ctx_sb[:])

                # second scores half of next panel
                if p + 1 < NPANEL:
                    ns1 = scores_half(p + 1, 1)
                    s_half = [ns0, ns1]

    nc.compile()
    return nc


def _get_nc():
    global _cached_nc
    if _cached_nc is None:
        _cached_nc = _build()
    return _cached_nc


last_exec_time_ns = None


def kernel(**inputs):
    global last_exec_time_ns
    nc = _get_nc()
    query = np.asarray(inputs["query"], dtype=np.float32)
    key = np.asarray(inputs["key"], dtype=np.float32)
    value = np.asarray(inputs["value"], dtype=np.float32)
    mask = np.asarray(inputs["mask"], dtype=np.int32)
    Wq = np.ascontiguousarray(np.asarray(inputs["Wq"], dtype=np.float32))
    bq = np.asarray(inputs["bq"], dtype=np.float32).reshape(1, D)
    Wk = np.ascontiguousarray(np.asarray(inputs["Wk"], dtype=np.float32))
    bk = np.asarray(inputs["bk"], dtype=np.float32).reshape(1, D)
    Wv = np.ascontiguousarray(np.asarray(inputs["Wv"], dtype=np.float32))
    bv = np.asarray(inputs["bv"], dtype=np.float32).reshape(1, D)

    in_maps = []
    for b in range(B):
        in_maps.append({
            "query": np.ascontiguousarray(query[b]),
            "key": np.ascontiguousarray(key[b]),
            "value": np.ascontiguousarray(value[b]),
            "mask": np.ascontiguousarray(mask[b].reshape(1, KN)),
            "Wq": Wq, "bq": bq, "Wk": Wk, "bk": bk, "Wv": Wv, "bv": bv,
        })

    trace = bool(os.environ.get("ATTN_TRACE"))
    try:
        res = run_bass_kernel_spmd(nc, in_maps, core_ids=list(range(N_CORES)),
                                   trace=trace)
    except Exception:
        if not trace:
            raise
        res = run_bass_kernel_spmd(nc, in_maps, core_ids=list(range(N_CORES)),
                                   trace=False)
    last_exec_time_ns = res.exec_time_ns

    ctx = np.stack([res.results[b]["ctx"] for b in range(B)])
    attn = np.stack([res.results[b]["attn"] for b in range(B)])
    return (ctx, attn)


# revision 12
# speedup vs baseline: 1.1310x; 1.1310x over previous
"""Fused attention layer (projections + masked softmax + context) on 8 TRN2
NeuronCores, data-parallel over batch (one batch element per core).

Per core (batch b):
  pq = q @ Wq.T + bq ; pk = k @ Wk.T + bk ; pv = v @ Wv.T + bv
  scores = pq @ pk.T / 32 ;  E = exp(scores) * mask
  attn = E / rowsum(E) ;  ctx = attn @ pv

Matmuls contract over the SBUF partition dim, so activations/weights are
transposed on-chip via TensorE (identity matmul). Projection matmuls run in
float32r (full PE rate for 4-byte data at free-dim >= 256); the scores and
context matmuls run in bf16 (operands produced by the projection epilogues,
so the casts are free). Step order q -> v -> k lets pk^T land directly in
its resident phase-2 tile with no DRAM round trip; pq^T is staged through
DRAM scratch in bf16 and re-read per 128-row query panel. Phase 2 is
software-pipelined: scores of panel p+1 run on TensorE while softmax of
panel p runs on ScalarE/VectorE.
"""
import os
from contextlib import ExitStack

import numpy as np

import concourse.bass as bass
import concourse.tile as tile
from concourse import bacc, mybir, masks
from concourse.bass_utils import run_bass_kernel_spmd

F32 = mybir.dt.float32
F32R = mybir.dt.float32r
BF16 = mybir.dt.bfloat16
I32 = mybir.dt.int32
AF = mybir.ActivationFunctionType
ALU = mybir.AluOpType

P = 128
B = 8
QN = 2048
KN = 2048
D = 1024           # DIN == DPROJ
DC = D // P        # 8 contraction chunks
NPANEL = QN // P   # 16 query panels
NS = KN // P       # 16 key chunks
SCALE = 1.0 / 32.0 # 1/sqrt(DPROJ)

N_CORES = 8

_cached_nc = None


def _build():
    nc = bacc.Bacc("TRN2", target_bir_lowering=False, debug=False,
                   num_devices=N_CORES)

    q_d = nc.dram_tensor("query", [QN, D], F32, kind="ExternalInput").ap()
    k_d = nc.dram_tensor("key", [KN, D], F32, kind="ExternalInput").ap()
    v_d = nc.dram_tensor("value", [KN, D], F32, kind="ExternalInput").ap()
    m_d = nc.dram_tensor("mask", [1, KN], I32, kind="ExternalInput").ap()
    w_q = nc.dram_tensor("Wq", [D, D], F32, kind="ExternalInput").ap()
    b_q = nc.dram_tensor("bq", [1, D], F32, kind="ExternalInput").ap()
    w_k = nc.dram_tensor("Wk", [D, D], F32, kind="ExternalInput").ap()
    b_k = nc.dram_tensor("bk", [1, D], F32, kind="ExternalInput").ap()
    w_v = nc.dram_tensor("Wv", [D, D], F32, kind="ExternalInput").ap()
    b_v = nc.dram_tensor("bv", [1, D], F32, kind="ExternalInput").ap()

    ctx_d = nc.dram_tensor("ctx", [QN, D], F32, kind="ExternalOutput").ap()
    attn_d = nc.dram_tensor("attn", [QN, KN], F32, kind="ExternalOutput").ap()

    # DRAM scratch for pq^T (bf16): [dc, d_in_dc, n]
    pqt_d = nc.dram_tensor("pqt_scratch", [DC, P, QN], BF16, kind="Internal").ap()

    cb_toggle = [0]  # alternate ACT/DVE for PSUM->SBUF copybacks

    def copyback(dst, src):
        if cb_toggle[0] % 2 == 0:
            nc.scalar.activation(dst, src, AF.Copy)
        else:
            nc.vector.tensor_copy(dst, src)
        cb_toggle[0] += 1

    with tile.TileContext(nc, pool_alloc_mode="queue") as tc, ExitStack() as top:
        const_pool = top.enter_context(tc.tile_pool(name="consts", bufs=1))
        ident = const_pool.tile([P, P], F32)
        masks.make_identity(nc, ident[:])
        ident_bf = const_pool.tile([P, P], BF16)
        nc.vector.tensor_copy(ident_bf[:], ident[:])
        zero_bias = const_pool.tile([P, 1], F32)
        nc.vector.memset(zero_bias[:], 0.0)
        bias_q = const_pool.tile([P, DC], F32)
        nc.gpsimd.dma_start(out=bias_q[:], in_=bass.AP(
            tensor=b_q.tensor, offset=b_q.offset, ap=[[1, P], [P, DC]]))
        bias_k = const_pool.tile([P, DC], F32)
        nc.gpsimd.dma_start(out=bias_k[:], in_=bass.AP(
            tensor=b_k.tensor, offset=b_k.offset, ap=[[1, P], [P, DC]]))

        def transpose_x(x_dram, xT, xnat_pool, ps_tr):
            """x [2048, 1024] natural -> xT [c_in_cc, cc, rows] f32r."""
            for s4 in range(4):  # 512-row panels
                xnat = xnat_pool.tile([P, 4, D], F32, tag="xnat")
                nc.sync.dma_start(
                    out=xnat[:],
                    in_=x_dram[s4 * 512:(s4 + 1) * 512, :]
                        .rearrange("(s p) c -> p s c", p=P))
                for cc in range(DC):
                    pt = ps_tr.tile([P, 4, P], F32, tag="pt")
                    for j in range(4):
                        nc.tensor.transpose(
                            pt[:, j, :], xnat[:, j, cc * P:(cc + 1) * P], ident[:])
                    copyback(xT[:, cc, s4 * 512:(s4 + 1) * 512], pt[:, :, :])

        def weights_T(w_dram, wt, ps_tr):
            """W [d, c] natural -> wt [c_in_cc, cc, d] f32r."""
            with tc.tile_pool(name="wnat", bufs=1) as wnp:
                w_nat = wnp.tile([P, DC, D], F32)
                nc.sync.dma_start(
                    out=w_nat[:],
                    in_=w_dram.rearrange("(dc p) c -> p dc c", p=P))
                for cc in range(DC):
                    for dg in range(2):
                        pt = ps_tr.tile([P, 4, P], F32, tag="pt")
                        for j in range(4):
                            dc = dg * 4 + j
                            nc.tensor.transpose(
                                pt[:, j, :],
                                w_nat[:, dc, cc * P:(cc + 1) * P], ident[:])
                        copyback(wt[:, cc, dg * 512:(dg + 1) * 512], pt[:, :, :])

        # ---- q step: pq^T -> DRAM scratch (bf16) ----
        with ExitStack() as ph:
            xnat_pool = ph.enter_context(tc.tile_pool(name="xnat_q", bufs=2))
            ps_tr = ph.enter_context(tc.tile_pool(name="ps_tr_q", bufs=2, space="PSUM"))
            ps_pr = ph.enter_context(tc.tile_pool(name="ps_pr_q", bufs=3, space="PSUM"))
            xtp = ph.enter_context(tc.tile_pool(name="xt_q", bufs=1))
            xT = xtp.tile([P, DC, QN], F32R)
            transpose_x(q_d, xT, xnat_pool, ps_tr)
            wtp = ph.enter_context(tc.tile_pool(name="wt_q", bufs=1))
            wt = wtp.tile([P, DC, D], F32R)
            weights_T(w_q, wt, ps_tr)
            obp = ph.enter_context(tc.tile_pool(name="ob_q", bufs=2))
            for dc in range(DC):
                ps_a = ps_pr.tile([P, 2, 512], F32, tag="ps_proj")
                ps_b = ps_pr.tile([P, 2, 512], F32, tag="ps_proj")
                for cc in range(DC):
                    for n4 in range(4):
                        pt = ps_a if n4 < 2 else ps_b
                        nc.tensor.matmul(
                            pt[:, n4 % 2, :],
                            lhsT=wt[:, cc, dc * P:(dc + 1) * P],
                            rhs=xT[:, cc, n4 * 512:(n4 + 1) * 512],
                            start=(cc == 0), stop=(cc == DC - 1))
                ob = obp.tile([P, 4, 512], BF16)
                for n4 in range(4):
                    pt = ps_a if n4 < 2 else ps_b
                    nc.scalar.activation(
                        ob[:, n4, :], pt[:, n4 % 2, :], AF.Identity,
                        bias=bias_q[:, dc:dc + 1], scale=1.0)
                nc.sync.dma_start(out=pqt_d[dc, :, :], in_=ob[:, :, :])

        # pv and pkt live through phase 2
        ph2 = top.enter_context(ExitStack())
        pv_pool = ph2.enter_context(tc.tile_pool(name="pv", bufs=1))
        pkt_pool = ph2.enter_context(tc.tile_pool(name="pkt", bufs=1))

        # ---- v step: pv [k_in_s, s, d] bf16, resident ----
        with ExitStack() as ph:
            bvp = ph.enter_context(tc.tile_pool(name="bv", bufs=1))
            bv_bc = bvp.tile([P, D], F32)
            nc.gpsimd.dma_start(out=bv_bc[:], in_=b_v.to_broadcast([P, D]))
            xnat_pool = ph.enter_context(tc.tile_pool(name="xnat_v", bufs=2))
            ps_tr = ph.enter_context(tc.tile_pool(name="ps_tr_v", bufs=2, space="PSUM"))
            ps_pr = ph.enter_context(tc.tile_pool(name="ps_pr_v", bufs=3, space="PSUM"))
            wtp = ph.enter_context(tc.tile_pool(name="wt_v", bufs=1))
            wvt = wtp.tile([P, DC, D], F32R)
            weights_T(w_v, wvt, ps_tr)
            xtp = ph.enter_context(tc.tile_pool(name="vt", bufs=1))
            vT = xtp.tile([P, DC, KN], F32R)
            transpose_x(v_d, vT, xnat_pool, ps_tr)

            pv = pv_pool.tile([P, NS, D], BF16)
            for s in range(NS):
                ps_v = ps_pr.tile([P, 2, 512], F32, tag="ps_proj")
                for cc in range(DC):
                    for d2 in range(2):
                        nc.tensor.matmul(
                            ps_v[:, d2, :],
                            lhsT=vT[:, cc, s * P:(s + 1) * P],
                            rhs=wvt[:, cc, d2 * 512:(d2 + 1) * 512],
                            start=(cc == 0), stop=(cc == DC - 1))
                for d2 in range(2):
                    nc.vector.scalar_tensor_tensor(
                        out=pv[:, s, d2 * 512:(d2 + 1) * 512],
                        in0=ps_v[:, d2, :], scalar=1.0,
                        in1=bv_bc[:, d2 * 512:(d2 + 1) * 512],
                        op0=ALU.mult, op1=ALU.add)

        # ---- k step: pk^T [d_in_dc, dc, n] bf16, resident ----
        with ExitStack() as ph:
            xnat_pool = ph.enter_context(tc.tile_pool(name="xnat_k", bufs=2))
            ps_tr = ph.enter_context(tc.tile_pool(name="ps_tr_k", bufs=2, space="PSUM"))
            ps_pr = ph.enter_context(tc.tile_pool(name="ps_pr_k", bufs=3, space="PSUM"))
            wtp = ph.enter_context(tc.tile_pool(name="wt_k", bufs=1))
            wkt = wtp.tile([P, DC, D], F32R)
            weights_T(w_k, wkt, ps_tr)
            xtp = ph.enter_context(tc.tile_pool(name="kt", bufs=1))
            kT = xtp.tile([P, DC, KN], F32R)
            transpose_x(k_d, kT, xnat_pool, ps_tr)

            pkt = pkt_pool.tile([P, DC, KN], BF16)
            for dc in range(DC):
                ps_a = ps_pr.tile([P, 2, 512], F32, tag="ps_proj")
                ps_b = ps_pr.tile([P, 2, 512], F32, tag="ps_proj")
                for cc in range(DC):
                    for n4 in range(4):
                        pt = ps_a if n4 < 2 else ps_b
                        nc.tensor.matmul(
                            pt[:, n4 % 2, :],
                            lhsT=wkt[:, cc, dc * P:(dc + 1) * P],
                            rhs=kT[:, cc, n4 * 512:(n4 + 1) * 512],
                            start=(cc == 0), stop=(cc == DC - 1))
                for n4 in range(4):
                    pt = ps_a if n4 < 2 else ps_b
                    nc.scalar.activation(
                        pkt[:, dc, n4 * 512:(n4 + 1) * 512],
                        pt[:, n4 % 2, :], AF.Identity,
                        bias=bias_k[:, dc:dc + 1], scale=1.0)

        # ---------------- Phase 2: attention, per query panel ----------------
        with ExitStack() as ph:
            maskp = ph.enter_context(tc.tile_pool(name="maskp", bufs=1))
            mask_f = maskp.tile([P, KN], BF16)
            with tc.tile_pool(name="mtmp", bufs=1) as mtmp:
                mask_i = mtmp.tile([P, KN], I32)
                nc.gpsimd.dma_start(out=mask_i[:], in_=m_d.to_broadcast([P, KN]))
                nc.vector.tensor_copy(mask_f[:], mask_i[:])

            pqp_pool = ph.enter_context(tc.tile_pool(name="pqp", bufs=3))
            e_pool = ph.enter_context(tc.tile_pool(name="em", bufs=2))
            attn_pool = ph.enter_context(tc.tile_pool(name="attnsb", bufs=2))
            at_pool = ph.enter_context(tc.tile_pool(name="attnT", bufs=2))
            ctx_pool = ph.enter_context(tc.tile_pool(name="ctxsb", bufs=2))
            small = ph.enter_context(tc.tile_pool(name="small", bufs=4))
            ps_s = ph.enter_context(tc.tile_pool(name="ps_s", bufs=2, space="PSUM"))
            ps_t = ph.enter_context(tc.tile_pool(name="ps_t", bufs=2, space="PSUM"))
            ps_c = ph.enter_context(tc.tile_pool(name="ps_c", bufs=1, space="PSUM"))

            pqp = [None] * NPANEL

            def load_pqp(p):
                pqp[p] = pqp_pool.tile([P, DC, P], BF16, name="pqp")
                nc.sync.dma_start(
                    out=pqp[p][:],
                    in_=pqt_d[:, :, p * P:(p + 1) * P].rearrange("a p n -> p a n"))

            def scores_half(p, h):
                ps = ps_s.tile([P, 2, 512], F32)
                for dc in range(DC):
                    for n2 in range(2):
                        nc.tensor.matmul(
                            ps[:, n2, :],
                            lhsT=pqp[p][:, dc, :],
                            rhs=pkt[:, dc, (2 * h + n2) * 512:(2 * h + n2 + 1) * 512],
                            start=(dc == 0), stop=(dc == DC - 1))
                return ps

            load_pqp(0)
            load_pqp(1)
            s_half = [scores_half(0, 0), scores_half(0, 1)]

            for p in range(NPANEL):
                # softmax for panel p (ACT/DVE; overlaps next scores on PE)
                em = e_pool.tile([P, KN], BF16)
                for h in range(2):
                    nc.scalar.activation(
                        em[:, h * 1024:(h + 1) * 1024],
                        s_half[h][:, :, :], AF.Exp,
                        bias=zero_bias[:], scale=SCALE)
                den = small.tile([P, 1], F32)
                nc.vector.scalar_tensor_tensor(
                    out=em[:], in0=em[:], scalar=1.0, in1=mask_f[:],
                    op0=ALU.mult, op1=ALU.mult, accum_out=den[:])
                rden = small.tile([P, 1], F32)
                nc.vector.reciprocal(rden[:], den[:])

                if p + 1 < NPANEL:
                    if p + 2 < NPANEL:
                        load_pqp(p + 2)
                    ns0 = scores_half(p + 1, 0)

                # attn output row-panel (f32)
                attn_sb = attn_pool.tile([P, KN], F32)
                nc.vector.tensor_scalar_mul(attn_sb[:], em[:], rden[:])
                nc.sync.dma_start(
                    out=attn_d[p * P:(p + 1) * P, :], in_=attn_sb[:])

                # transpose Em (bf16) -> attnT [k_in_s, s, nq]; the 1/den
                # normalization is folded into the ctx copyback scale
                attnT = at_pool.tile([P, NS, P], BF16)
                for g in range(4):
                    pt = ps_t.tile([P, 4, P], BF16)
                    for j in range(4):
                        s_idx = g * 4 + j
                        nc.tensor.transpose(
                            pt[:, j, :],
                            em[:, s_idx * P:(s_idx + 1) * P], ident_bf[:])
                    copyback(attnT[:, g * 4:(g + 1) * 4, :], pt[:, :, :])

                # ctx panel
                ps_ctx = ps_c.tile([P, 2, 512], F32)
                for s in range(NS):
                    for d2 in range(2):
                        nc.tensor.matmul(
                            ps_ctx[:, d2, :],
                            lhsT=attnT[:, s, :],
                            rhs=pv[:, s, d2 * 512:(d2 + 1) * 512],
                            start=(s == 0), stop=(s == NS - 1))
                ctx_sb = ctx_pool.tile([P, D], F32)
                nc.scalar.activation(ctx_sb[:], ps_ctx[:, :, :], AF.Copy,
                                     scale=rden[:])
                nc.sync.dma_start(
                    out=ctx_d[p * P:(p + 1) * P, :], in_=ctx_sb[:])

                if p + 1 < NPANEL:
                    ns1 = scores_half(p + 1, 1)
                    s_half = [ns0, ns1]

    nc.compile()
    return nc


def _get_nc():
    global _cached_nc
    if _cached_nc is None:
        _cached_nc = _build()
    return _cached_nc


last_exec_time_ns = None


def kernel(**inputs):
    global last_exec_time_ns
    nc = _get_nc()
    query = np.asarray(inputs["query"], dtype=np.float32)
    key = np.asarray(inputs["key"], dtype=np.float32)
    value = np.asarray(inputs["value"], dtype=np.float32)
    mask = np.asarray(inputs["mask"], dtype=np.int32)
    Wq = np.ascontiguousarray(np.asarray(inputs["Wq"], dtype=np.float32))
    bq = np.asarray(inputs["bq"], dtype=np.float32).reshape(1, D)
    Wk = np.ascontiguousarray(np.asarray(inputs["Wk"], dtype=np.float32))
    bk = np.asarray(inputs["bk"], dtype=np.float32).reshape(1, D)
    Wv = np.ascontiguousarray(np.asarray(inputs["Wv"], dtype=np.float32))
    bv = np.asarray(inputs["bv"], dtype=np.float32).reshape(1, D)

    in_maps = []
    for b in range(B):
        in_maps.append({
            "query": np.ascontiguousarray(query[b]),
            "key": np.ascontiguousarray(key[b]),
            "value": np.ascontiguousarray(value[b]),
            "mask": np.ascontiguousarray(mask[b].reshape(1, KN)),
            "Wq": Wq, "bq": bq, "Wk": Wk, "bk": bk, "Wv": Wv, "bv": bv,
        })

    trace = bool(os.environ.get("ATTN_TRACE"))
    try:
        res = run_bass_kernel_spmd(nc, in_maps, core_ids=list(range(N_CORES)),
                                   trace=trace)
    except Exception:
        if not trace:
            raise
        res = run_bass_kernel_spmd(nc, in_maps, core_ids=list(range(N_CORES)),
                                   trace=False)
    last_exec_time_ns = res.exec_time_ns

    ctx = np.stack([res.results[b]["ctx"] for b in range(B)])
    attn = np.stack([res.results[b]["attn"] for b in range(B)])
    return (ctx, attn)
